# revision 8
# baseline (speedup 1.0000x reference)
"""Mixture-of-Depths routing kernel for 8 Trainium2 NeuronCores.

Sharding: core c handles batch b = c//2, sequence half q = c%2 (2048 tokens),
with the FFN tensor-parallel over the hidden dim within each core pair
(w1 column-sharded, w2 row-sharded). Router logits are computed per half and
exchanged with a pair AllGather; FFN partial outputs are summed with a pair
AllReduce. Top-k selection is exact: kth_largest returns the 512th-largest
logit, a >=threshold mask is compacted to sorted indices with sparse_gather,
selected rows are gathered/scattered with indirect DMA.
"""
import sys

for _p in ("/opt/trn_rl_repo", "/root/.axon_site/_ro/trn_rl_repo"):
    if _p not in sys.path:
        sys.path.append(_p)

import numpy as np
import concourse.bass as bass
import concourse.bacc as bacc
import concourse.mybir as mybir
import concourse.tile as tile
from concourse.bass_utils import run_bass_kernel_spmd

P = 128
B, T, C, H = 4, 4096, 1024, 4096
K = 512                # tokens selected per batch
HT = T // 2            # tokens per core (half sequence)
HH = H // 2            # hidden per core (tensor parallel)
NXT = HT // P          # 16 x tiles per core
f32 = mybir.dt.float32
f32r = mybir.dt.float32r
i32 = mybir.dt.int32
u32 = mybir.dt.uint32
AF = mybir.ActivationFunctionType
ALU = mybir.AluOpType
PAIRS = [[0, 1], [2, 3], [4, 5], [6, 7]]


def build():
    nc = bacc.Bacc("TRN2", target_bir_lowering=False, debug=False, num_devices=8)

    xb = nc.dram_tensor("xb", [T, C], f32, kind="ExternalInput").ap()
    xbh = nc.dram_tensor("xbh", [HT, C], f32, kind="ExternalInput").ap()
    w1h = nc.dram_tensor("w1h", [C, HH], f32, kind="ExternalInput").ap()
    w2h = nc.dram_tensor("w2h", [HH, C], f32, kind="ExternalInput").ap()
    wmod_r = nc.dram_tensor("wmod_r", [P, C], f32, kind="ExternalInput").ap()
    wmlp_r = nc.dram_tensor("wmlp_r", [P, C], f32, kind="ExternalInput").ap()
    qoff = nc.dram_tensor("qoff", [1, 1], f32, kind="ExternalInput").ap()

    out_half = nc.dram_tensor("out_half", [HT + 1, C], f32, kind="ExternalOutput").ap()
    lg_out = nc.dram_tensor("lg_out", [1, T], f32, kind="ExternalOutput").ap()
    th_out = nc.dram_tensor("th_out", [1, 2], f32, kind="ExternalOutput").ap()
    m_out = nc.dram_tensor("m_out", [1, HT], f32, kind="ExternalOutput").ap()
    sp_out = nc.dram_tensor("sp_out", [P, 1], f32, kind="ExternalOutput").ap()

    scr_i = nc.dram_tensor("scr_i", [2 * K], i32).ap()     # idx | dest roundtrip
    scr_w = nc.dram_tensor("scr_w", [K], f32).ap()         # weights roundtrip

    with tile.TileContext(nc) as tc:
        from contextlib import ExitStack
        with ExitStack() as ctx:
            const = ctx.enter_context(tc.tile_pool(name="const", bufs=1))
            xpool = ctx.enter_context(tc.tile_pool(name="xpool", bufs=1))
            stat = ctx.enter_context(tc.tile_pool(name="stat", bufs=1))
            selp = ctx.enter_context(tc.tile_pool(name="selp", bufs=1))
            xselp = ctx.enter_context(tc.tile_pool(name="xselp", bufs=1))
            big = ctx.enter_context(tc.tile_pool(name="big", bufs=1))
            wpool = ctx.enter_context(tc.tile_pool(name="wpool", bufs=3))
            ypool = ctx.enter_context(tc.tile_pool(name="ypool", bufs=2))
            drp = ctx.enter_context(tc.tile_pool(name="drp", bufs=1, space="DRAM"))
            tp_ps = ctx.enter_context(tc.tile_pool(name="tp_ps", bufs=2, space="PSUM"))
            mm_ps = ctx.enter_context(tc.tile_pool(name="mm_ps", bufs=1, space="PSUM"))

            # ---- constants
            wmod_t = const.tile([P, C], f32)
            wmlp_t = const.tile([P, C], f32)
            nc.sync.dma_start(wmod_t[:], wmod_r)
            nc.sync.dma_start(wmlp_t[:], wmlp_r)
            from concourse.masks import make_identity
            ident = const.tile([P, P], f32)
            make_identity(nc, ident[:])
            neg1 = const.tile([16, T // 16], f32)
            nc.vector.memset(neg1[:], -1.0)
            trash = const.tile([16, K // 16], f32)
            nc.vector.memset(trash[:], float(HT))
            qoff_t = const.tile([1, 1], f32)
            nc.sync.dma_start(qoff_t[:], qoff)
            qoff16 = const.tile([16, 1], f32)
            nc.gpsimd.partition_broadcast(qoff16[:], qoff_t[:1, :1], channels=16)

            # ---- phase 1: stream x half; logits; copy rows to out
            xt = []
            lgh = stat.tile([P, NXT], f32)
            prod = stat.tile([P, C], f32)
            copies = []
            for i in range(NXT):
                t = xpool.tile([P, C], f32, tag=f"xt{i}", name=f"xt{i}")
                xt.append(t)
                nc.sync.dma_start(t[:], xbh[i * P:(i + 1) * P, :])
                nc.vector.scalar_tensor_tensor(
                    out=prod[:], in0=t[:], scalar=1.0, in1=wmod_t[:],
                    op0=ALU.mult, op1=ALU.mult, accum_out=lgh[:, i:i + 1])
                ci = nc.sync.dma_start(out_half[i * P:(i + 1) * P, :], t[:])
                copies.append(ci)

            # ---- phase 2: exchange logit halves (pair AllGather)
            lg_lin = drp.tile([2048], f32)
            lg_full = drp.tile([2, 2048], f32)
            nc.sync.dma_start(lg_lin[:].rearrange("(i p) -> p i", p=P), lgh[:])
            nc.gpsimd.collective_compute(
                "AllGather", ALU.bypass, replica_groups=PAIRS,
                ins=[lg_lin.opt()], outs=[lg_full.opt()])
            nc.sync.dma_start(lg_out.rearrange("a b -> (a b)"), lg_full[:].rearrange("a b -> (a b)"))

            # ---- phase 3: threshold + selection
            kin = selp.tile([P, T // P], f32)
            nc.sync.dma_start(kin[:], lg_full[:].rearrange("a (y p) -> p (a y)", p=P))
            kout = selp.tile([1, 2], f32)
            nc.gpsimd.kth_largest(kout[:], kin[:], n_per_lane=T // P, k=510,
                                  quantile=1.0 - 510.5 / (T - 1))
            nc.sync.dma_start(th_out, kout[:])
            th16 = selp.tile([16, 1], f32)
            nc.gpsimd.partition_broadcast(th16[:], kout[:1, 1:2], channels=16)

            FW = T // 16  # 256
            lgw = selp.tile([16, FW], f32)
            nc.sync.dma_start(lgw[:], lg_full[:].rearrange("a (g p) -> p (a g)", p=16))
            io = selp.tile([16, FW], i32)
            nc.gpsimd.iota(io[:], pattern=[[16, FW]], base=0, channel_multiplier=1)
            iof = selp.tile([16, FW], f32)
            nc.vector.tensor_copy(iof[:], io[:])
            mask = selp.tile([16, FW], i32)
            nc.vector.tensor_scalar(mask[:], lgw[:], th16[:, 0:1], None, ALU.is_ge)
            selv = selp.tile([16, FW], f32)
            nc.vector.select(selv[:], mask[:], iof[:], neg1[:])
            selw_in = selp.tile([16, FW], f32)
            nc.vector.select(selw_in[:], mask[:], lgw[:], neg1[:])

            KW = K // 16  # 32
            idx_f = selp.tile([16, KW], f32)
            nf1 = selp.tile([1, 1], u32)
            nc.gpsimd.sparse_gather(idx_f[:], selv[:], num_found=nf1[:])
            wsel = selp.tile([16, KW], f32)
            nf2 = selp.tile([1, 1], u32)
            nc.gpsimd.sparse_gather(wsel[:], selw_in[:], num_found=nf2[:])

            # dest rows: local = idx - q*HT clamped to trash row HT
            dst_f = selp.tile([16, KW], f32)
            nc.vector.tensor_scalar(dst_f[:], idx_f[:], qoff16[:, 0:1], None,
                                    ALU.subtract)
            dst_c = selp.tile([16, KW], f32)
            nc.vector.tensor_scalar_min(dst_c[:], dst_f[:], float(HT))
            neg_m = selp.tile([16, KW], i32)
            nc.vector.tensor_scalar(neg_m[:], dst_f[:], 0.0, None, ALU.is_lt)
            dst_s = selp.tile([16, KW], f32)
            nc.vector.select(dst_s[:], neg_m[:], trash[:], dst_c[:])

            idx_i = selp.tile([16, KW], i32)
            nc.vector.tensor_copy(idx_i[:], idx_f[:])
            dst_i = selp.tile([16, KW], i32)
            nc.vector.tensor_copy(dst_i[:], dst_s[:])

            # roundtrip through DRAM to relayout (16,KW)->(128,K/128)
            nc.sync.dma_start(scr_i[0:K].rearrange("(f p) -> p f", p=16), idx_i[:])
            nc.sync.dma_start(scr_i[K:2 * K].rearrange("(f p) -> p f", p=16), dst_i[:])
            nc.sync.dma_start(scr_w[:].rearrange("(f p) -> p f", p=16), wsel[:])
            NKC = K // P  # 4
            idx128 = selp.tile([P, NKC], i32)
            dst128 = selp.tile([P, NKC], i32)
            w128 = selp.tile([P, NKC], f32)
            nc.sync.dma_start(idx128[:], scr_i[0:K].rearrange("(c p) -> p c", p=P))
            nc.sync.dma_start(dst128[:], scr_i[K:2 * K].rearrange("(c p) -> p c", p=P))
            nc.sync.dma_start(w128[:], scr_w[:].rearrange("(c p) -> p c", p=P))

            # ---- phase 5: gather selected rows (full batch indices into xb)
            xsel = []
            for c in range(NKC):
                g = xselp.tile([P, C], f32, tag=f"xs{c}", name=f"xs{c}")
                xsel.append(g)
                nc.gpsimd.indirect_dma_start(
                    out=g[:], out_offset=None, in_=xb,
                    in_offset=bass.IndirectOffsetOnAxis(ap=idx128[:, c:c + 1], axis=0))

            # ---- phase 6: transpose -> xselT (c partitions, K tokens free)
            NCC = C // P  # 8
            xselT = big.tile([P, NCC * K], f32r, tag="xselT")
            for c in range(NKC):
                for kc in range(NCC):
                    pt = tp_ps.tile([P, P], f32, space="PSUM", tag="tp")
                    nc.tensor.transpose(pt[:], xsel[c][:, kc * P:(kc + 1) * P], ident[:])
                    nc.vector.tensor_copy(
                        xselT[:, kc * K + c * P: kc * K + (c + 1) * P], pt[:])

            # ---- phase 7a: h = gelu(xsel @ w1h)  -> (h_lo, [hi, tok]) f32r
            NHI = HH // P  # 16
            h_sb = big.tile([P, NHI * K], f32r, tag="h_sb")
            NG = 4  # hi per psum group
            for g in range(NHI // NG):
                pts = []
                for j in range(NG):
                    pts.append(mm_ps.tile([P, K], f32, space="PSUM", tag=f"mm{j}", name=f"mm1g{g}_{j}"))
                for kc in range(NCC):
                    wt = wpool.tile([P, NG * P], f32r, tag="w1t")
                    nc.gpsimd.dma_start(
                        wt[:], w1h[kc * P:(kc + 1) * P,
                                   g * NG * P:(g + 1) * NG * P])
                    for j in range(NG):
                        nc.tensor.matmul(
                            pts[j][:], wt[:, j * P:(j + 1) * P],
                            xselT[:, kc * K:(kc + 1) * K],
                            start=(kc == 0), stop=(kc == NCC - 1))
                for j in range(NG):
                    hi = g * NG + j
                    nc.scalar.activation(h_sb[:, hi * K:(hi + 1) * K], pts[j][:],
                                         AF.Gelu_apprx_tanh)

            # ---- phase 7b: ypart = h @ w2h -> (tok, C) partial
            y_dram = drp.tile([K, C], f32)
            ysum = drp.tile([K, C], f32)
            for cc in range(2):
                pts = []
                for t4 in range(NKC):
                    pts.append(mm_ps.tile([P, 512], f32, space="PSUM", tag=f"mm{t4}", name=f"mm2c{cc}_{t4}"))
                for hi in range(NHI):
                    wt2 = wpool.tile([P, 512], f32r, tag="w2t")
                    nc.gpsimd.dma_start(
                        wt2[:], w2h[hi * P:(hi + 1) * P, cc * 512:(cc + 1) * 512])
                    for t4 in range(NKC):
                        nc.tensor.matmul(
                            pts[t4][:],
                            h_sb[:, hi * K + t4 * P: hi * K + (t4 + 1) * P],
                            wt2[:], start=(hi == 0), stop=(hi == NHI - 1))
                for t4 in range(NKC):
                    yp = ypool.tile([P, 512], f32, tag="yp")
                    nc.vector.tensor_copy(yp[:], pts[t4][:])
                    nc.sync.dma_start(
                        y_dram[t4 * P:(t4 + 1) * P, cc * 512:(cc + 1) * 512], yp[:])

            nc.gpsimd.collective_compute(
                "AllReduce", ALU.add, replica_groups=PAIRS,
                ins=[y_dram.opt()], outs=[ysum.opt()])

            # ---- phase 8: u = xsel + ysum * weight ; scatter to out rows
            for t4 in range(NKC):
                ys = ypool.tile([P, C], f32, tag="ys")
                nc.sync.dma_start(ys[:], ysum[t4 * P:(t4 + 1) * P, :])
                u = ypool.tile([P, C], f32, tag="u")
                nc.vector.scalar_tensor_tensor(
                    out=u[:], in0=ys[:], scalar=w128[:, t4:t4 + 1], in1=xsel[t4][:],
                    op0=ALU.mult, op1=ALU.add)
                si = nc.gpsimd.indirect_dma_start(
                    out=out_half,
                    out_offset=bass.IndirectOffsetOnAxis(ap=dst128[:, t4:t4 + 1], axis=0),
                    in_=u[:], in_offset=None)
                for ci in copies:
                    tile.add_dep_helper(si.ins, ci.ins, sync=True,
                                        reason="scatter after phase1 row copies")

            # ---- phase 9: aux (m logits, softplus) — off critical path
            mh = stat.tile([P, NXT], f32)
            prod2 = stat.tile([P, C], f32)
            for i in range(NXT):
                nc.vector.scalar_tensor_tensor(
                    out=prod2[:], in0=xt[i][:], scalar=1.0, in1=wmlp_t[:],
                    op0=ALU.mult, op1=ALU.mult, accum_out=mh[:, i:i + 1])
            nc.sync.dma_start(m_out.rearrange("a (i p) -> (a p) i", p=P), mh[:])
            ab = stat.tile([P, NXT], f32)
            nc.scalar.activation(ab[:], mh[:], AF.Abs)
            ex = stat.tile([P, NXT], f32)
            nc.scalar.activation(ex[:], ab[:], AF.Exp, scale=-1.0)
            l1p = stat.tile([P, NXT], f32)
            nc.scalar.activation(l1p[:], ex[:], AF.Ln, bias=1.0)
            rl = stat.tile([P, NXT], f32)
            nc.vector.tensor_scalar_max(rl[:], mh[:], 0.0)
            sp = stat.tile([P, NXT], f32)
            nc.vector.tensor_add(sp[:], rl[:], l1p[:])
            spr = stat.tile([P, 1], f32)
            nc.vector.tensor_reduce(spr[:], sp[:], mybir.AxisListType.X, ALU.add)
            nc.sync.dma_start(sp_out, spr[:])

    nc.compile()
    return nc


_NC = None
_LAST_IN_MAPS = None


def kernel(x, w_mod, w_mlp, w1, w2):
    global _NC
    x = np.ascontiguousarray(np.asarray(x, np.float32))
    w_mod = np.asarray(w_mod, np.float32)
    w_mlp = np.asarray(w_mlp, np.float32)
    w1 = np.ascontiguousarray(np.asarray(w1, np.float32))
    w2 = np.ascontiguousarray(np.asarray(w2, np.float32))
    if _NC is None:
        _NC = build()

    wmod_r = np.ascontiguousarray(np.broadcast_to(w_mod[:, 0], (P, C)))
    wmlp_r = np.ascontiguousarray(np.broadcast_to(w_mlp[:, 0], (P, C)))
    in_maps = []
    for c in range(8):
        b, q = c // 2, c % 2
        in_maps.append({
            "xb": x[b],
            "xbh": x[b, q * HT:(q + 1) * HT],
            "w1h": np.ascontiguousarray(w1[:, q * HH:(q + 1) * HH]),
            "w2h": np.ascontiguousarray(w2[q * HH:(q + 1) * HH, :]),
            "wmod_r": wmod_r,
            "wmlp_r": wmlp_r,
            "qoff": np.array([[q * HT]], np.float32),
        })
    global _LAST_IN_MAPS
    _LAST_IN_MAPS = in_maps
    res = run_bass_kernel_spmd(_NC, in_maps, core_ids=list(range(8)))

    out = np.empty((B, T, C), np.float32)
    sp_total = 0.0
    lg = {}
    th = {}
    m0 = {}
    for c in range(8):
        r = res.results[c]
        b, q = c // 2, c % 2
        out[b, q * HT:(q + 1) * HT] = r["out_half"][:HT]
        sp_total += float(r["sp_out"].astype(np.float64).sum())
        if q == 0:
            lg[b] = r["lg_out"][0]
            th[b] = float(r["th_out"][0, 1])
        if b == 0:
            m0[q] = r["m_out"][0]

    union = np.zeros(T, bool)
    for b in range(B):
        union |= lg[b] >= th[b]
    m_flat = np.concatenate([m0[0], m0[1]])
    aux = (sp_total - float(m_flat[union].astype(np.float64).sum())) / (B * T)
    return out, np.float32(aux)


# revision 11
# speedup vs baseline: 1.1089x; 1.1089x over previous
"""Mixture-of-Depths routing kernel for 8 Trainium2 NeuronCores.

Sharding: core c handles batch b = c//2, sequence half q = c%2 (2048 tokens),
with the FFN tensor-parallel over the hidden dim within each core pair
(w1 column-sharded, w2 row-sharded). Router logits are computed per half and
exchanged with a pair AllGather; FFN partial outputs are summed with a pair
AllReduce. Top-k selection is exact: kth_largest returns the 512th-largest
logit, a >=threshold mask is compacted to sorted indices with sparse_gather,
selected rows are gathered/scattered with indirect DMA.
"""
import sys

for _p in ("/opt/trn_rl_repo", "/root/.axon_site/_ro/trn_rl_repo"):
    if _p not in sys.path:
        sys.path.append(_p)

import numpy as np
import concourse.bass as bass
import concourse.bacc as bacc
import concourse.mybir as mybir
import concourse.tile as tile
from concourse.bass_utils import run_bass_kernel_spmd

P = 128
B, T, C, H = 4, 4096, 1024, 4096
K = 512                # tokens selected per batch
HT = T // 2            # tokens per core (half sequence)
HH = H // 2            # hidden per core (tensor parallel)
NXT = HT // P          # 16 x tiles per core
f32 = mybir.dt.float32
f32r = mybir.dt.float32r
i32 = mybir.dt.int32
u32 = mybir.dt.uint32
AF = mybir.ActivationFunctionType
ALU = mybir.AluOpType
PAIRS = [[0, 1], [2, 3], [4, 5], [6, 7]]


def build():
    nc = bacc.Bacc("TRN2", target_bir_lowering=False, debug=False, num_devices=8)

    xb = nc.dram_tensor("xb", [T, C], f32, kind="ExternalInput").ap()
    xbh = nc.dram_tensor("xbh", [HT, C], f32, kind="ExternalInput").ap()
    w1h = nc.dram_tensor("w1h", [C, HH], f32r, kind="ExternalInput").ap()
    w2h = nc.dram_tensor("w2h", [HH, C], f32r, kind="ExternalInput").ap()
    wmod_r = nc.dram_tensor("wmod_r", [P, C], f32, kind="ExternalInput").ap()
    wmlp_r = nc.dram_tensor("wmlp_r", [P, C], f32, kind="ExternalInput").ap()
    qoff = nc.dram_tensor("qoff", [1, 1], f32, kind="ExternalInput").ap()

    out_half = nc.dram_tensor("out_half", [HT + 1, C], f32, kind="ExternalOutput").ap()
    lg_out = nc.dram_tensor("lg_out", [1, T], f32, kind="ExternalOutput").ap()
    th_out = nc.dram_tensor("th_out", [1, 2], f32, kind="ExternalOutput").ap()
    m_out = nc.dram_tensor("m_out", [1, HT], f32, kind="ExternalOutput").ap()
    sp_out = nc.dram_tensor("sp_out", [P, 1], f32, kind="ExternalOutput").ap()

    scr_w = nc.dram_tensor("scr_w", [K], f32).ap()         # weights roundtrip

    with tile.TileContext(nc) as tc:
        from contextlib import ExitStack
        with ExitStack() as ctx:
            const = ctx.enter_context(tc.tile_pool(name="const", bufs=1))
            xpool = ctx.enter_context(tc.tile_pool(name="xpool", bufs=1))
            stat = ctx.enter_context(tc.tile_pool(name="stat", bufs=1))
            selp = ctx.enter_context(tc.tile_pool(name="selp", bufs=1))
            xselp = ctx.enter_context(tc.tile_pool(name="xselp", bufs=1))
            big = ctx.enter_context(tc.tile_pool(name="big", bufs=1))
            wpool = ctx.enter_context(tc.tile_pool(name="wpool", bufs=3))
            ypool = ctx.enter_context(tc.tile_pool(name="ypool", bufs=2))
            drp = ctx.enter_context(tc.tile_pool(name="drp", bufs=1, space="DRAM"))
            tp_ps = ctx.enter_context(tc.tile_pool(name="tp_ps", bufs=2, space="PSUM"))
            mm_ps = ctx.enter_context(tc.tile_pool(name="mm_ps", bufs=1, space="PSUM"))

            # ---- constants
            wmod_t = const.tile([P, C], f32)
            wmlp_t = const.tile([P, C], f32)
            nc.sync.dma_start(wmod_t[:], wmod_r)
            nc.sync.dma_start(wmlp_t[:], wmlp_r)
            from concourse.masks import make_identity
            ident = const.tile([P, P], f32)
            make_identity(nc, ident[:])
            neg1 = const.tile([16, T // 16], f32)
            nc.vector.memset(neg1[:], -1.0)
            trash = const.tile([16, K // 16], f32)
            nc.vector.memset(trash[:], float(HT))
            qoff_t = const.tile([1, 1], f32)
            nc.sync.dma_start(qoff_t[:], qoff)
            qoff16 = const.tile([16, 1], f32)
            nc.gpsimd.partition_broadcast(qoff16[:], qoff_t[:1, :1], channels=16)

            # ---- phase 1: stream x half; logits; copy rows to out
            xt = []
            lgh = stat.tile([P, NXT], f32)
            prod = stat.tile([P, C], f32)
            copies = []
            for i in range(NXT):
                t = xpool.tile([P, C], f32, tag=f"xt{i}", name=f"xt{i}")
                xt.append(t)
                nc.sync.dma_start(t[:], xbh[i * P:(i + 1) * P, :])
                nc.vector.scalar_tensor_tensor(
                    out=prod[:], in0=t[:], scalar=1.0, in1=wmod_t[:],
                    op0=ALU.mult, op1=ALU.mult, accum_out=lgh[:, i:i + 1])
                ci = nc.sync.dma_start(out_half[i * P:(i + 1) * P, :], t[:])
                copies.append(ci)

            # ---- phase 2: exchange logit halves (pair AllGather)
            lg_lin = drp.tile([2048], f32)
            lg_full = drp.tile([2, 2048], f32)
            nc.sync.dma_start(lg_lin[:].rearrange("(i p) -> p i", p=P), lgh[:])
            nc.gpsimd.collective_compute(
                "AllGather", ALU.bypass, replica_groups=PAIRS,
                ins=[lg_lin.opt()], outs=[lg_full.opt()])
            nc.sync.dma_start(lg_out.rearrange("a b -> (a b)"), lg_full[:].rearrange("a b -> (a b)"))

            # ---- phase 3: threshold + selection
            kin = selp.tile([P, T // P], f32)
            nc.sync.dma_start(kin[:], lg_full[:].rearrange("a (y p) -> p (a y)", p=P))
            kout = selp.tile([1, 2], f32)
            nc.gpsimd.kth_largest(kout[:], kin[:], n_per_lane=T // P, k=510,
                                  quantile=1.0 - 510.5 / (T - 1))
            nc.sync.dma_start(th_out, kout[:])
            th16 = selp.tile([16, 1], f32)
            nc.gpsimd.partition_broadcast(th16[:], kout[:1, 1:2], channels=16)

            FW = T // 16  # 256
            lgw = selp.tile([16, FW], f32)
            nc.sync.dma_start(lgw[:], lg_full[:].rearrange("a (g p) -> p (a g)", p=16))
            io = selp.tile([16, FW], i32)
            nc.gpsimd.iota(io[:], pattern=[[16, FW]], base=0, channel_multiplier=1)
            iof = selp.tile([16, FW], f32)
            nc.vector.tensor_copy(iof[:], io[:])
            mask = selp.tile([16, FW], i32)
            nc.vector.tensor_scalar(mask[:], lgw[:], th16[:, 0:1], None, ALU.is_ge)
            selv = selp.tile([16, FW], f32)
            nc.vector.select(selv[:], mask[:], iof[:], neg1[:])
            selw_in = selp.tile([16, FW], f32)
            nc.vector.select(selw_in[:], mask[:], lgw[:], neg1[:])

            KW = K // 16  # 32
            idx_f = selp.tile([16, KW], f32)
            nf1 = selp.tile([1, 1], u32)
            nc.gpsimd.sparse_gather(idx_f[:], selv[:], num_found=nf1[:])
            wsel = selp.tile([16, KW], f32)
            nf2 = selp.tile([1, 1], u32)
            nc.gpsimd.sparse_gather(wsel[:], selw_in[:], num_found=nf2[:])

            # dest rows: local = idx - q*HT clamped to trash row HT
            dst_f = selp.tile([16, KW], f32)
            nc.vector.tensor_scalar(dst_f[:], idx_f[:], qoff16[:, 0:1], None,
                                    ALU.subtract)
            dst_c = selp.tile([16, KW], f32)
            nc.vector.tensor_scalar_min(dst_c[:], dst_f[:], float(HT))
            neg_m = selp.tile([16, KW], i32)
            nc.vector.tensor_scalar(neg_m[:], dst_f[:], 0.0, None, ALU.is_lt)
            dst_s = selp.tile([16, KW], f32)
            nc.vector.select(dst_s[:], neg_m[:], trash[:], dst_c[:])

            i16 = mybir.dt.int16
            idx16 = selp.tile([P, KW], i16)
            nc.vector.tensor_copy(idx16[:16, :], idx_f[:])
            dst16 = selp.tile([P, KW], i16)
            nc.vector.tensor_copy(dst16[:16, :], dst_s[:])
            for k in range(1, 8):
                nc.sync.dma_start(idx16[16 * k:16 * (k + 1), :], idx16[:16, :])
                nc.sync.dma_start(dst16[16 * k:16 * (k + 1), :], dst16[:16, :])
            # weights to (128, K/128) layout via DRAM roundtrip
            nc.sync.dma_start(scr_w[:].rearrange("(f p) -> p f", p=16), wsel[:])
            NKC = K // P  # 4
            w128 = selp.tile([P, NKC], f32)
            nc.sync.dma_start(w128[:], scr_w[:].rearrange("(c p) -> p c", p=P))

            # ---- phase 5: gather selected rows: xsel[p, c, :] = xb[sel[c*128+p]]
            xsel_t = xselp.tile([P, NKC * C], f32, tag="xsel", name="xsel_t")
            xsel3 = xsel_t[:].rearrange("p (c e) -> p c e", e=C)
            nc.gpsimd.dma_gather(
                out_ap=xsel3, in_ap=xb, idxs_ap=idx16[:], num_idxs=K,
                num_idxs_reg=K, elem_size=C)
            xsel = [xsel_t[:, c * C:(c + 1) * C] for c in range(NKC)]

            # ---- phase 6: transpose -> xselT (c partitions, K tokens free)
            NCC = C // P  # 8
            xselT = big.tile([P, NCC * K], f32r, tag="xselT")
            for c in range(NKC):
                for kc in range(NCC):
                    pt = tp_ps.tile([P, P], f32, space="PSUM", tag="tp")
                    nc.tensor.transpose(pt[:], xsel[c][:, kc * P:(kc + 1) * P], ident[:])
                    nc.vector.tensor_copy(
                        xselT[:, kc * K + c * P: kc * K + (c + 1) * P], pt[:])

            # ---- phase 7a: h = gelu(xsel @ w1h)  -> (h_lo, [hi, tok]) f32r
            NHI = HH // P  # 16
            h_sb = big.tile([P, NHI * K], f32r, tag="h_sb")
            NG = 4  # hi per psum group
            for g in range(NHI // NG):
                pts = []
                for j in range(NG):
                    pts.append(mm_ps.tile([P, K], f32, space="PSUM", tag=f"mm{j}", name=f"mm1g{g}_{j}"))
                for kc in range(NCC):
                    wt = wpool.tile([P, NG * P], f32r, tag="w1t")
                    nc.sync.dma_start(
                        wt[:], w1h[kc * P:(kc + 1) * P,
                                   g * NG * P:(g + 1) * NG * P])
                    for j in range(NG):
                        nc.tensor.matmul(
                            pts[j][:], wt[:, j * P:(j + 1) * P],
                            xselT[:, kc * K:(kc + 1) * K],
                            start=(kc == 0), stop=(kc == NCC - 1))
                for j in range(NG):
                    hi = g * NG + j
                    nc.scalar.activation(h_sb[:, hi * K:(hi + 1) * K], pts[j][:],
                                         AF.Gelu_apprx_tanh)

            # ---- phase 7b: ypart = h @ w2h -> (tok, C) partial
            y_dram = drp.tile([K, C], f32)
            ysum = drp.tile([K, C], f32)
            for cc in range(2):
                pts = []
                for t4 in range(NKC):
                    pts.append(mm_ps.tile([P, 512], f32, space="PSUM", tag=f"mm{t4}", name=f"mm2c{cc}_{t4}"))
                for hi in range(NHI):
                    wt2 = wpool.tile([P, 512], f32r, tag="w2t")
                    nc.sync.dma_start(
                        wt2[:], w2h[hi * P:(hi + 1) * P, cc * 512:(cc + 1) * 512])
                    for t4 in range(NKC):
                        nc.tensor.matmul(
                            pts[t4][:],
                            h_sb[:, hi * K + t4 * P: hi * K + (t4 + 1) * P],
                            wt2[:], start=(hi == 0), stop=(hi == NHI - 1))
                for t4 in range(NKC):
                    yp = ypool.tile([P, 512], f32, tag="yp")
                    nc.vector.tensor_scalar(yp[:], pts[t4][:], w128[:, t4:t4 + 1],
                                            None, ALU.mult)
                    nc.sync.dma_start(
                        y_dram[t4 * P:(t4 + 1) * P, cc * 512:(cc + 1) * 512], yp[:])

            nc.gpsimd.collective_compute(
                "AllReduce", ALU.add, replica_groups=PAIRS,
                ins=[y_dram.opt()], outs=[ysum.opt()])

            # ---- phase 8: load weighted pair-summed y; scatter-add into out rows
            ysc_t = ypool.tile([P, NKC * C], f32, tag="ysc", name="ysc_t")
            nc.sync.dma_start(
                ysc_t[:].rearrange("p (c e) -> p c e", e=C),
                ysum[:].rearrange("(c p) e -> p c e", p=P))
            si = nc.gpsimd.dma_scatter_add(
                out_ap=out_half, in_ap=ysc_t[:].rearrange("p (c e) -> p c e", e=C),
                idxs_ap=dst16[:], num_idxs=K, num_idxs_reg=K, elem_size=C)
            for ci in copies:
                tile.add_dep_helper(si.ins, ci.ins, sync=True,
                                    reason="scatter-add after phase1 row copies")

            # ---- phase 9: aux (m logits, softplus) — off critical path
            mh = stat.tile([P, NXT], f32)
            prod2 = stat.tile([P, C], f32)
            for i in range(NXT):
                nc.vector.scalar_tensor_tensor(
                    out=prod2[:], in0=xt[i][:], scalar=1.0, in1=wmlp_t[:],
                    op0=ALU.mult, op1=ALU.mult, accum_out=mh[:, i:i + 1])
            nc.sync.dma_start(m_out.rearrange("a (i p) -> (a p) i", p=P), mh[:])
            ab = stat.tile([P, NXT], f32)
            nc.scalar.activation(ab[:], mh[:], AF.Abs)
            ex = stat.tile([P, NXT], f32)
            nc.scalar.activation(ex[:], ab[:], AF.Exp, scale=-1.0)
            l1p = stat.tile([P, NXT], f32)
            nc.scalar.activation(l1p[:], ex[:], AF.Ln, bias=1.0)
            rl = stat.tile([P, NXT], f32)
            nc.vector.tensor_scalar_max(rl[:], mh[:], 0.0)
            sp = stat.tile([P, NXT], f32)
            nc.vector.tensor_add(sp[:], rl[:], l1p[:])
            spr = stat.tile([P, 1], f32)
            nc.vector.tensor_reduce(spr[:], sp[:], mybir.AxisListType.X, ALU.add)
            nc.sync.dma_start(sp_out, spr[:])

    nc.compile()
    return nc


_NC = None
_LAST_IN_MAPS = None


def kernel(x, w_mod, w_mlp, w1, w2):
    global _NC
    x = np.ascontiguousarray(np.asarray(x, np.float32))
    w_mod = np.asarray(w_mod, np.float32)
    w_mlp = np.asarray(w_mlp, np.float32)
    w1 = np.ascontiguousarray(np.asarray(w1, np.float32))
    w2 = np.ascontiguousarray(np.asarray(w2, np.float32))
    if _NC is None:
        _NC = build()

    wmod_r = np.ascontiguousarray(np.broadcast_to(w_mod[:, 0], (P, C)))
    wmlp_r = np.ascontiguousarray(np.broadcast_to(w_mlp[:, 0], (P, C)))
    in_maps = []
    for c in range(8):
        b, q = c // 2, c % 2
        in_maps.append({
            "xb": x[b],
            "xbh": x[b, q * HT:(q + 1) * HT],
            "w1h": np.ascontiguousarray(w1[:, q * HH:(q + 1) * HH]),
            "w2h": np.ascontiguousarray(w2[q * HH:(q + 1) * HH, :]),
            "wmod_r": wmod_r,
            "wmlp_r": wmlp_r,
            "qoff": np.array([[q * HT]], np.float32),
        })
    global _LAST_IN_MAPS
    _LAST_IN_MAPS = in_maps
    res = run_bass_kernel_spmd(_NC, in_maps, core_ids=list(range(8)))

    out = np.empty((B, T, C), np.float32)
    sp_total = 0.0
    lg = {}
    th = {}
    m0 = {}
    for c in range(8):
        r = res.results[c]
        b, q = c // 2, c % 2
        out[b, q * HT:(q + 1) * HT] = r["out_half"][:HT]
        sp_total += float(r["sp_out"].astype(np.float64).sum())
        if q == 0:
            lg[b] = r["lg_out"][0]
            th[b] = float(r["th_out"][0, 1])
        if b == 0:
            m0[q] = r["m_out"][0]

    union = np.zeros(T, bool)
    for b in range(B):
        union |= lg[b] >= th[b]
    m_flat = np.concatenate([m0[0], m0[1]])
    aux = (sp_total - float(m_flat[union].astype(np.float64).sum())) / (B * T)
    return out, np.float32(aux)


# revision 14
# speedup vs baseline: 2.2296x; 2.0107x over previous
"""Mixture-of-Depths routing kernel for 8 Trainium2 NeuronCores.

Sharding: core c handles batch b = c//2, sequence half q = c%2 (2048 tokens),
with the FFN tensor-parallel over the hidden dim within each core pair
(w1 column-sharded, w2 row-sharded). Router logits are computed per half and
exchanged with a pair AllGather; FFN partial outputs are summed with a pair
AllReduce. Top-k selection is exact: kth_largest returns the 512th-largest
logit, a >=threshold mask is compacted to sorted indices with sparse_gather,
selected rows are gathered/scattered with indirect DMA.
"""
import sys

for _p in ("/opt/trn_rl_repo", "/root/.axon_site/_ro/trn_rl_repo"):
    if _p not in sys.path:
        sys.path.append(_p)

import numpy as np
import concourse.bass as bass
import concourse.bacc as bacc
import concourse.mybir as mybir
import concourse.tile as tile
from concourse.bass_utils import run_bass_kernel_spmd

P = 128
B, T, C, H = 4, 4096, 1024, 4096
K = 512                # tokens selected per batch
HT = T // 2            # tokens per core (half sequence)
HH = H // 2            # hidden per core (tensor parallel)
NXT = HT // P          # 16 x tiles per core
f32 = mybir.dt.float32
f32r = mybir.dt.float32r
i32 = mybir.dt.int32
u32 = mybir.dt.uint32
AF = mybir.ActivationFunctionType
ALU = mybir.AluOpType
PAIRS = [[0, 1], [2, 3], [4, 5], [6, 7]]


def build():
    nc = bacc.Bacc("TRN2", target_bir_lowering=False, debug=False, num_devices=8)

    xb = nc.dram_tensor("xb", [T, C], f32, kind="ExternalInput").ap()
    xbh = nc.dram_tensor("xbh", [HT, C], f32, kind="ExternalInput").ap()
    w1h = nc.dram_tensor("w1h", [C, HH], f32r, kind="ExternalInput").ap()
    w2h = nc.dram_tensor("w2h", [HH, C], f32r, kind="ExternalInput").ap()
    wmod_r = nc.dram_tensor("wmod_r", [P, C], f32, kind="ExternalInput").ap()
    wmlp_r = nc.dram_tensor("wmlp_r", [P, C], f32, kind="ExternalInput").ap()
    qoff = nc.dram_tensor("qoff", [1, 1], f32, kind="ExternalInput").ap()

    out_half = nc.dram_tensor("out_half", [HT + 1, C], f32, kind="ExternalOutput").ap()
    lg_out = nc.dram_tensor("lg_out", [1, T], f32, kind="ExternalOutput").ap()
    th_out = nc.dram_tensor("th_out", [1, 1], f32, kind="ExternalOutput").ap()
    m_out = nc.dram_tensor("m_out", [1, HT], f32, kind="ExternalOutput").ap()
    sp_out = nc.dram_tensor("sp_out", [P, 1], f32, kind="ExternalOutput").ap()

    scr_w = nc.dram_tensor("scr_w", [K], f32).ap()         # weights roundtrip

    with tile.TileContext(nc) as tc:
        from contextlib import ExitStack
        with ExitStack() as ctx:
            const = ctx.enter_context(tc.tile_pool(name="const", bufs=1))
            xpool = ctx.enter_context(tc.tile_pool(name="xpool", bufs=4))
            stat = ctx.enter_context(tc.tile_pool(name="stat", bufs=1))
            selp = ctx.enter_context(tc.tile_pool(name="selp", bufs=1))
            xselp = ctx.enter_context(tc.tile_pool(name="xselp", bufs=1))
            big = ctx.enter_context(tc.tile_pool(name="big", bufs=1))
            wpool = ctx.enter_context(tc.tile_pool(name="wpool", bufs=3))
            ypool = ctx.enter_context(tc.tile_pool(name="ypool", bufs=2))
            drp = ctx.enter_context(tc.tile_pool(name="drp", bufs=1, space="DRAM"))
            tp_ps = ctx.enter_context(tc.tile_pool(name="tp_ps", bufs=2, space="PSUM"))
            mm_ps = ctx.enter_context(tc.tile_pool(name="mm_ps", bufs=1, space="PSUM"))

            # ---- constants
            wmod_t = const.tile([P, C], f32)
            wmlp_t = const.tile([P, C], f32)
            nc.sync.dma_start(wmod_t[:], wmod_r)
            nc.sync.dma_start(wmlp_t[:], wmlp_r)
            from concourse.masks import make_identity
            ident = const.tile([P, P], f32)
            make_identity(nc, ident[:])
            neg1 = const.tile([16, T // 16], f32)
            nc.vector.memset(neg1[:], -1.0)
            trash = const.tile([16, K // 16], f32)
            nc.vector.memset(trash[:], float(HT))
            qoff_t = const.tile([1, 1], f32)
            nc.sync.dma_start(qoff_t[:], qoff)
            qoff16 = const.tile([16, 1], f32)
            nc.gpsimd.partition_broadcast(qoff16[:], qoff_t[:1, :1], channels=16)

            # ---- phase 1: stream x half; logits + aux logits; copy rows out
            lgh = stat.tile([P, NXT], f32)
            mh = stat.tile([P, NXT], f32)
            prod = stat.tile([P, C], f32)
            prod2 = stat.tile([P, C], f32)
            copies = []
            for i in range(NXT):
                t = xpool.tile([P, C], f32, tag="xt", name=f"xt{i}")
                nc.sync.dma_start(t[:], xbh[i * P:(i + 1) * P, :])
                nc.vector.scalar_tensor_tensor(
                    out=prod[:], in0=t[:], scalar=1.0, in1=wmod_t[:],
                    op0=ALU.mult, op1=ALU.mult, accum_out=lgh[:, i:i + 1])
                nc.vector.scalar_tensor_tensor(
                    out=prod2[:], in0=t[:], scalar=1.0, in1=wmlp_t[:],
                    op0=ALU.mult, op1=ALU.mult, accum_out=mh[:, i:i + 1])
                ci = nc.sync.dma_start(out_half[i * P:(i + 1) * P, :], t[:])
                copies.append(ci)

            # ---- phase 2: exchange logit halves (pair AllGather)
            lg_lin = drp.tile([2048], f32)
            lg_full = drp.tile([2, 2048], f32)
            nc.sync.dma_start(lg_lin[:].rearrange("(i p) -> p i", p=P), lgh[:])
            nc.gpsimd.collective_compute(
                "AllGather", ALU.bypass, replica_groups=PAIRS,
                ins=[lg_lin.opt()], outs=[lg_full.opt()])
            nc.sync.dma_start(lg_out.rearrange("a b -> (a b)"), lg_full[:].rearrange("a b -> (a b)"))

            # ---- phase 3: exact 512th-largest via 3-round 128-way search
            from concourse import bass_isa
            lg_row = selp.tile([1, T], f32)
            nc.sync.dma_start(lg_row[:], lg_full[:].rearrange("a b -> (a b)")[None, :])
            ones_col = selp.tile([1, P], f32)
            nc.vector.memset(ones_col[:], 1.0)
            lgrep = big.tile([P, T], f32, tag="lgrep")
            for nb in range(T // 512):
                rp = tp_ps.tile([P, 512], f32, space="PSUM", tag="tp",
                                name=f"rep{nb}")
                nc.tensor.matmul(rp[:], ones_col[:, :], lg_row[:1, nb * 512:(nb + 1) * 512],
                                 start=True, stop=True)
                nc.vector.tensor_copy(lgrep[:, nb * 512:(nb + 1) * 512], rp[:])
            pvec_i = selp.tile([P, 1], i32)
            nc.gpsimd.iota(pvec_i[:], pattern=[[1, 1]], base=0, channel_multiplier=1)
            pvec = selp.tile([P, 1], f32)
            nc.vector.tensor_copy(pvec[:], pvec_i[:])
            thlo = selp.tile([P, 1], f32)
            nc.vector.memset(thlo[:], -8.0)
            junk = big.tile([P, T], f32, tag="junk")
            counts = selp.tile([P, 1], f32)
            c2 = selp.tile([P, 1], f32)
            svec = selp.tile([P, 1], f32)
            for r in range(3):
                step = 16.0 / (128.0 ** (r + 1))
                thp = selp.tile([P, 1], f32, tag="thp", name=f"thp{r}")
                nc.vector.scalar_tensor_tensor(
                    out=thp[:], in0=pvec[:], scalar=float(step), in1=thlo[:],
                    op0=ALU.mult, op1=ALU.add)
                nc.vector.tensor_scalar(junk[:], lgrep[:], thp[:, 0:1], 0.0,
                                        ALU.is_ge, ALU.add, accum_out=counts[:])
                nc.vector.tensor_scalar(c2[:], counts[:], 511.5, None, ALU.is_ge)
                nc.gpsimd.partition_all_reduce(svec[:], c2[:], channels=P,
                                               reduce_op=bass_isa.ReduceOp.add)
                nc.vector.scalar_tensor_tensor(
                    out=thlo[:], in0=svec[:], scalar=float(step), in1=thlo[:],
                    op0=ALU.mult, op1=ALU.add)
                nc.vector.tensor_scalar_add(thlo[:], thlo[:], -float(step))
            nc.sync.dma_start(th_out, thlo[:1, 0:1])
            th16 = thlo[:16, :]

            FW = T // 16  # 256
            lgw = selp.tile([16, FW], f32)
            nc.sync.dma_start(lgw[:], lg_full[:].rearrange("a (g p) -> p (a g)", p=16))
            io = selp.tile([16, FW], i32)
            nc.gpsimd.iota(io[:], pattern=[[16, FW]], base=0, channel_multiplier=1)
            iof = selp.tile([16, FW], f32)
            nc.vector.tensor_copy(iof[:], io[:])
            mask = selp.tile([16, FW], i32)
            nc.vector.tensor_scalar(mask[:], lgw[:], th16[:, 0:1], None, ALU.is_ge)
            selv = selp.tile([16, FW], f32)
            nc.vector.select(selv[:], mask[:], iof[:], neg1[:])
            selw_in = selp.tile([16, FW], f32)
            nc.vector.select(selw_in[:], mask[:], lgw[:], neg1[:])

            KW = K // 16  # 32
            idx_f = selp.tile([16, KW], f32)
            nf1 = selp.tile([1, 1], u32)
            nc.gpsimd.sparse_gather(idx_f[:], selv[:], num_found=nf1[:])
            wsel = selp.tile([16, KW], f32)
            nf2 = selp.tile([1, 1], u32)
            nc.gpsimd.sparse_gather(wsel[:], selw_in[:], num_found=nf2[:])

            # dest rows: local = idx - q*HT clamped to trash row HT
            dst_f = selp.tile([16, KW], f32)
            nc.vector.tensor_scalar(dst_f[:], idx_f[:], qoff16[:, 0:1], None,
                                    ALU.subtract)
            dst_c = selp.tile([16, KW], f32)
            nc.vector.tensor_scalar_min(dst_c[:], dst_f[:], float(HT))
            neg_m = selp.tile([16, KW], i32)
            nc.vector.tensor_scalar(neg_m[:], dst_f[:], 0.0, None, ALU.is_lt)
            dst_s = selp.tile([16, KW], f32)
            nc.vector.select(dst_s[:], neg_m[:], trash[:], dst_c[:])

            i16 = mybir.dt.int16
            idx16 = selp.tile([P, KW], i16)
            nc.vector.tensor_copy(idx16[:16, :], idx_f[:])
            dst16 = selp.tile([P, KW], i16)
            nc.vector.tensor_copy(dst16[:16, :], dst_s[:])
            for k in range(1, 8):
                nc.sync.dma_start(idx16[16 * k:16 * (k + 1), :], idx16[:16, :])
                nc.sync.dma_start(dst16[16 * k:16 * (k + 1), :], dst16[:16, :])
            # weights to (128, K/128) layout via DRAM roundtrip
            nc.sync.dma_start(scr_w[:].rearrange("(f p) -> p f", p=16), wsel[:])
            NKC = K // P  # 4
            w128 = selp.tile([P, NKC], f32)
            nc.sync.dma_start(w128[:], scr_w[:].rearrange("(c p) -> p c", p=P))

            # ---- phase 5: gather selected rows: xsel[p, c, :] = xb[sel[c*128+p]]
            xsel_t = xselp.tile([P, NKC * C], f32, tag="xsel", name="xsel_t")
            xsel3 = xsel_t[:].rearrange("p (c e) -> p c e", e=C)
            nc.gpsimd.dma_gather(
                out_ap=xsel3, in_ap=xb, idxs_ap=idx16[:], num_idxs=K,
                num_idxs_reg=K, elem_size=C)
            xsel = [xsel_t[:, c * C:(c + 1) * C] for c in range(NKC)]

            # ---- phase 6: transpose -> xselT (c partitions, K tokens free)
            NCC = C // P  # 8
            xselT = big.tile([P, NCC * K], f32r, tag="xselT")
            for c in range(NKC):
                for kc in range(NCC):
                    pt = tp_ps.tile([P, P], f32, space="PSUM", tag="tp")
                    nc.tensor.transpose(pt[:], xsel[c][:, kc * P:(kc + 1) * P], ident[:])
                    nc.vector.tensor_copy(
                        xselT[:, kc * K + c * P: kc * K + (c + 1) * P], pt[:])

            # ---- phase 7a: h = gelu(xsel @ w1h)  -> (h_lo, [hi, tok]) f32r
            NHI = HH // P  # 16
            h_sb = big.tile([P, NHI * K], f32r, tag="h_sb")
            NG = 4  # hi per psum group
            for g in range(NHI // NG):
                pts = []
                for j in range(NG):
                    pts.append(mm_ps.tile([P, K], f32, space="PSUM", tag=f"mm{j}", name=f"mm1g{g}_{j}"))
                for kc in range(NCC):
                    wt = wpool.tile([P, NG * P], f32r, tag="w1t")
                    nc.sync.dma_start(
                        wt[:], w1h[kc * P:(kc + 1) * P,
                                   g * NG * P:(g + 1) * NG * P])
                    for j in range(NG):
                        nc.tensor.matmul(
                            pts[j][:], wt[:, j * P:(j + 1) * P],
                            xselT[:, kc * K:(kc + 1) * K],
                            start=(kc == 0), stop=(kc == NCC - 1))
                for j in range(NG):
                    hi = g * NG + j
                    nc.scalar.activation(h_sb[:, hi * K:(hi + 1) * K], pts[j][:],
                                         AF.Gelu_apprx_tanh)

            # ---- phase 7b: ypart = h @ w2h -> (tok, C) partial
            y_dram = drp.tile([K, C], f32)
            ysum = drp.tile([K, C], f32)
            for cc in range(2):
                pts = []
                for t4 in range(NKC):
                    pts.append(mm_ps.tile([P, 512], f32, space="PSUM", tag=f"mm{t4}", name=f"mm2c{cc}_{t4}"))
                for hi in range(NHI):
                    wt2 = wpool.tile([P, 512], f32r, tag="w2t")
                    nc.sync.dma_start(
                        wt2[:], w2h[hi * P:(hi + 1) * P, cc * 512:(cc + 1) * 512])
                    for t4 in range(NKC):
                        nc.tensor.matmul(
                            pts[t4][:],
                            h_sb[:, hi * K + t4 * P: hi * K + (t4 + 1) * P],
                            wt2[:], start=(hi == 0), stop=(hi == NHI - 1))
                for t4 in range(NKC):
                    yp = ypool.tile([P, 512], f32, tag="yp")
                    nc.vector.tensor_scalar(yp[:], pts[t4][:], w128[:, t4:t4 + 1],
                                            None, ALU.mult)
                    nc.sync.dma_start(
                        y_dram[t4 * P:(t4 + 1) * P, cc * 512:(cc + 1) * 512], yp[:])

            nc.gpsimd.collective_compute(
                "AllReduce", ALU.add, replica_groups=PAIRS,
                ins=[y_dram.opt()], outs=[ysum.opt()])

            # ---- phase 8: load weighted pair-summed y; scatter-add into out rows
            ysc_t = ypool.tile([P, NKC * C], f32, tag="ysc", name="ysc_t")
            nc.sync.dma_start(
                ysc_t[:].rearrange("p (c e) -> p c e", e=C),
                ysum[:].rearrange("(c p) e -> p c e", p=P))
            si = nc.gpsimd.dma_scatter_add(
                out_ap=out_half, in_ap=ysc_t[:].rearrange("p (c e) -> p c e", e=C),
                idxs_ap=dst16[:], num_idxs=K, num_idxs_reg=K, elem_size=C)
            for ci in copies:
                tile.add_dep_helper(si.ins, ci.ins, sync=True,
                                    reason="scatter-add after phase1 row copies")

            # ---- phase 9: aux softplus — off critical path
            nc.sync.dma_start(m_out.rearrange("a (i p) -> (a p) i", p=P), mh[:])
            ab = stat.tile([P, NXT], f32)
            nc.scalar.activation(ab[:], mh[:], AF.Abs)
            ex = stat.tile([P, NXT], f32)
            nc.scalar.activation(ex[:], ab[:], AF.Exp, scale=-1.0)
            l1p = stat.tile([P, NXT], f32)
            nc.scalar.activation(l1p[:], ex[:], AF.Ln, bias=1.0)
            rl = stat.tile([P, NXT], f32)
            nc.vector.tensor_scalar_max(rl[:], mh[:], 0.0)
            sp = stat.tile([P, NXT], f32)
            nc.vector.tensor_add(sp[:], rl[:], l1p[:])
            spr = stat.tile([P, 1], f32)
            nc.vector.tensor_reduce(spr[:], sp[:], mybir.AxisListType.X, ALU.add)
            nc.sync.dma_start(sp_out, spr[:])

    nc.compile()
    return nc


_NC = None
_LAST_IN_MAPS = None


def kernel(x, w_mod, w_mlp, w1, w2):
    global _NC
    x = np.ascontiguousarray(np.asarray(x, np.float32))
    w_mod = np.asarray(w_mod, np.float32)
    w_mlp = np.asarray(w_mlp, np.float32)
    w1 = np.ascontiguousarray(np.asarray(w1, np.float32))
    w2 = np.ascontiguousarray(np.asarray(w2, np.float32))
    if _NC is None:
        _NC = build()

    wmod_r = np.ascontiguousarray(np.broadcast_to(w_mod[:, 0], (P, C)))
    wmlp_r = np.ascontiguousarray(np.broadcast_to(w_mlp[:, 0], (P, C)))
    in_maps = []
    for c in range(8):
        b, q = c // 2, c % 2
        in_maps.append({
            "xb": x[b],
            "xbh": x[b, q * HT:(q + 1) * HT],
            "w1h": np.ascontiguousarray(w1[:, q * HH:(q + 1) * HH]),
            "w2h": np.ascontiguousarray(w2[q * HH:(q + 1) * HH, :]),
            "wmod_r": wmod_r,
            "wmlp_r": wmlp_r,
            "qoff": np.array([[q * HT]], np.float32),
        })
    global _LAST_IN_MAPS
    _LAST_IN_MAPS = in_maps
    res = run_bass_kernel_spmd(_NC, in_maps, core_ids=list(range(8)))

    out = np.empty((B, T, C), np.float32)
    sp_total = 0.0
    lg = {}
    th = {}
    m0 = {}
    for c in range(8):
        r = res.results[c]
        b, q = c // 2, c % 2
        out[b, q * HT:(q + 1) * HT] = r["out_half"][:HT]
        sp_total += float(r["sp_out"].astype(np.float64).sum())
        if q == 0:
            lg[b] = r["lg_out"][0]
            th[b] = float(r["th_out"][0, 0])
        if b == 0:
            m0[q] = r["m_out"][0]

    union = np.zeros(T, bool)
    for b in range(B):
        union |= lg[b] >= th[b]
    m_flat = np.concatenate([m0[0], m0[1]])
    aux = (sp_total - float(m_flat[union].astype(np.float64).sum())) / (B * T)
    return out, np.float32(aux)
